# revision 32
# baseline (speedup 1.0000x reference)
"""Attention via the folded-kernel trick: scores = x (Wq Wk^T) x^T.

M^T = Wk Wq^T is precomputed host-side (exact algebra, weight-only), so
the device computes HT = M x^T in place of K^T and contracts it directly
against the raw x^T query slabs -- the whole Q-projection phase (and its
SBUF tile) disappears: per-core PE floor drops 143.5us -> 129.6us. V is
still deduplicated pairwise: each core computes V' for its own sequence
half and the pair exchanges halves with one 2-rank AllGather whose
deadline (out phase, ~105us) is far beyond the observed ~28us-barrier +
~23us-transfer collective.

Schedule notes (trace-derived):
- One flat tile scope with a single 8-deep PSUM rotation: no
  pool-transition barrier between HT and scores.
- 13 HAM-warmup matmuls cover the DMA-bound A1 prefix (~2MB of wv+xq at
  ~330GB/s lands ~14.5us); shorter warmups leave >1us PE gaps that
  re-throttle the HAM clock gate to 4/8 for ~3.4us (measured).
- Keeping A1 kc-major matters: a denser dc-major variant (8 concurrent
  PSUM banks, zero gaps) consistently tripped the chip-wide P0 power
  downclock (~2.0GHz for the WHOLE kernel, +23us).
- V' layout puts the softmax-denominator ones-column at position 0, so
  the output epilogue splits per PSUM bank: reciprocal + 510-col
  multiply + DMA overlap the 258-col run, leaving ~1us after the last
  matmul.
- Startup DMAs round-robin over the 3 DMA-capable rings (gpsimd/SP/ACT)
  in consumption order.
"""

import numpy as np

import concourse.bass as bass
import concourse.mybir as mybir
import concourse.tile as tile
from concourse import bacc
from concourse.bass_utils import run_bass_kernel_spmd

N_CORES = 8
B, N, D, OUT = 4, 2048, 768, 768
NQ = N // 2
P = 128
DC = D // P
KC = N // P
HKC = KC // 2  # k-chunks per half
F32 = mybir.dt.float32
FP16 = mybir.dt.float16
PAIRS = [[0, 1], [2, 3], [4, 5], [6, 7]]

VW = OUT + 2  # vp row: [ones, zero, V 0:768]
Q_BLOCKS = [(0, 512), (512, 512)]


def build_attention_nc():
    nc = bacc.Bacc("TRN2", target_bir_lowering=False, debug=False)
    xh = nc.dram_tensor("xh", [D, N], FP16, kind="ExternalInput")
    xq = nc.dram_tensor("xq", [D, NQ], FP16, kind="ExternalInput")
    w = nc.dram_tensor("w", [2, D, OUT], FP16, kind="ExternalInput")
    out = nc.dram_tensor("out", [NQ, OUT], F32, kind="ExternalOutput")

    with tile.TileContext(nc) as tc:
        with (
            tc.tile_pool(name="persist", bufs=1) as persist,
            tc.tile_pool(name="dpool", bufs=1, space="DRAM") as dpool,
        ):
            qslab_tiles = [
                persist.tile([P, DC, 512], FP16, name=f"qslab{s}")
                for s in range(2)
            ]
            # HT[d2,k] physical order; per-slab tiles so the scores phase
            # can start on slab 0 while slab 3's copies are still landing
            ht_tiles = [
                persist.tile([P, DC, 512], FP16, name=f"ht{s}")
                for s in range(4)
            ]
            vp = persist.tile([P, KC, VW], FP16)  # V' gather(=physical) order

            vpb_in = dpool.tile([P, HKC, VW], FP16)
            vpb_out = dpool.tile([2, P, HKC, VW], FP16)

            ones_sc = persist.tile([P, 1], F32, name="ones_sc")
            nc.vector.memset(ones_sc, 1.0)
            zero_sc = persist.tile([P, 1], F32, name="zero_sc")
            nc.vector.memset(zero_sc, 0.0)

            with (
                tc.tile_pool(name="slabs", bufs=4) as slabs,
                tc.tile_pool(name="psa", bufs=8, space="PSUM") as psa,
                tc.tile_pool(name="wpool", bufs=1) as wpool,
                tc.tile_pool(name="stage", bufs=4) as stage,
                tc.tile_pool(name="expp", bufs=34) as expp,
                tc.tile_pool(name="obp", bufs=6) as obp,
                tc.tile_pool(name="smallp", bufs=4) as smallp,
            ):
                wv_sb = wpool.tile([P, DC, OUT], FP16)
                m_sb = wpool.tile([P, DC, OUT], FP16)

                # HAM warmup while the first DMAs fly (sized so A1's first
                # run starts right as the last wv/qslab0 slices land)
                warm = wpool.tile([P, 512], FP16, name="warm")
                nc.vector.memset(warm, 1.0)
                wps = psa.tile([P, 512], F32, tag="psa", name="wps")
                for i in range(11):
                    nc.tensor.matmul(
                        wps, warm[:, 0:P], warm, start=(i == 0), stop=(i == 10)
                    )

                # DMAs: wv/xq-slab0 first (V feeds the gather, so it runs
                # earliest) round-robined over the 3 DMA-capable rings
                # (gpsimd/SP/ACT), then xq-slab1, M^T, and the 4 xh slabs
                rings = [nc.gpsimd, nc.sync, nc.scalar]
                ri = 0
                for dc in range(DC):
                    rings[ri % 3].dma_start(
                        out=wv_sb[:, dc, :],
                        in_=w[1][dc * P : (dc + 1) * P, :],
                    )
                    ri += 1
                    rings[ri % 3].dma_start(
                        out=qslab_tiles[0][:, dc, :],
                        in_=xq[dc * P : (dc + 1) * P, 0:512],
                    )
                    ri += 1
                for dc in range(DC):
                    rings[ri % 3].dma_start(
                        out=qslab_tiles[1][:, dc, :],
                        in_=xq[dc * P : (dc + 1) * P, 512:1024],
                    )
                    ri += 1
                for dc in range(DC):
                    m_ring = nc.gpsimd if dc % 2 == 0 else nc.scalar
                    m_ring.dma_start(
                        out=m_sb[:, dc, :], in_=w[0][dc * P : (dc + 1) * P, :]
                    )
                kslab_tiles = []
                for s in range(4):
                    kslab = slabs.tile(
                        [P, DC, 512], FP16, tag="slab", name=f"kslab{s}"
                    )
                    nc.sync.dma_start(
                        out=kslab,
                        in_=xh[:, s * 512 : (s + 1) * 512].rearrange(
                            "(dc p) n -> p dc n", p=P
                        ),
                    )
                    kslab_tiles.append(kslab)

                # ---- A1: V' half (earliest -> feeds the gather) ----
                # kc-PAIR-major: two k-chunks accumulate concurrently (4
                # PSUM banks), so each arriving (wv, qslab) dc-slice feeds
                # 0.64us of matmuls against the ~1.05us HBM-bound slice
                # cadence -- kc0 no longer serializes on the last dc slice
                # while kc1's data-free work waits. (Full dc-major with 8
                # banks tripped the chip-wide P0 downclock; this doesn't.)
                for s in range(2):
                    slab = qslab_tiles[s]
                    dc_order = list(range(DC))
                    for jp in range(2):
                        pv = [
                            psa.tile([P, 512], F32, tag="psa", name=f"pv{t}")
                            for t in range(4)
                        ]
                        for di, dc in enumerate(dc_order):
                            for u in range(2):
                                j = jp * 2 + u
                                # ps1/ps2 share the stationary slab slice,
                                # so the second weight load overlaps/elides
                                nc.tensor.matmul(
                                    pv[2 * u][:, 0:384],
                                    slab[:, dc, j * P : (j + 1) * P],
                                    wv_sb[:, dc, 0:384],
                                    start=(di == 0),
                                    stop=(di == DC - 1),
                                )
                                nc.tensor.matmul(
                                    pv[2 * u + 1][:, 0:384],
                                    slab[:, dc, j * P : (j + 1) * P],
                                    wv_sb[:, dc, 384:OUT],
                                    start=(di == 0),
                                    stop=(di == DC - 1),
                                )
                        for u in range(2):
                            kc = s * 4 + jp * 2 + u
                            vst = stage.tile([P, VW], FP16, tag="vst", bufs=9)
                            nc.vector.tensor_copy(vst[:, 0:1], ones_sc)
                            nc.vector.tensor_copy(vst[:, 1:2], zero_sc)
                            nc.vector.tensor_copy(
                                vst[:, 2:386], pv[2 * u][:, 0:384]
                            )
                            nc.vector.tensor_copy(
                                vst[:, 386:VW], pv[2 * u + 1][:, 0:384]
                            )
                            nc.gpsimd.dma_start(out=vpb_in[:, kc, :], in_=vst)
                nc.gpsimd.collective_compute(
                    "AllGather",
                    mybir.AluOpType.bypass,
                    replica_groups=PAIRS,
                    ins=[vpb_in.opt()],
                    outs=[vpb_out.opt()],
                )
                # NOT on the scalar ring: the ACT sequencer is busy with
                # the exp activations by the time the gather lands
                for h in range(2):
                    nc.sync.dma_start(
                        out=vp[:, h * HKC : (h + 1) * HKC, :], in_=vpb_out[h]
                    )

                # ---- A2: HT = M x^T, physical order, local ----
                for s in range(4):
                    slab = kslab_tiles[s]
                    for oc in range(DC):
                        ps = psa.tile([P, 512], F32, tag="psa")
                        for dc in range(DC):
                            nc.tensor.matmul(
                                ps,
                                m_sb[:, dc, oc * P : (oc + 1) * P],
                                slab[:, dc, :],
                                start=(dc == 0),
                                stop=(dc == DC - 1),
                            )
                        nc.vector.tensor_copy(ht_tiles[s][:, oc, :], ps)

                # ---- phase B: all scoresT runs, then all out runs ----
                # (same scope: PSUM tiles share the psa rotation, so no
                # pool-transition barrier between HT and scores)
                ets = {}
                for bi, (q0, qb) in enumerate(Q_BLOCKS):
                    qslab = qslab_tiles[bi]
                    for kc in range(KC):
                        hts = ht_tiles[kc // 4]
                        kr = kc % 4
                        st = psa.tile([P, 512], F32, tag="psa")
                        for dc in range(DC):
                            nc.tensor.matmul(
                                st,
                                hts[:, dc, kr * P : (kr + 1) * P],
                                qslab[:, dc, :],
                                start=(dc == 0),
                                stop=(dc == DC - 1),
                            )
                        et = expp.tile(
                            [P, 512], FP16, tag="exp", name=f"et{bi}_{kc}"
                        )
                        nc.scalar.activation(
                            et,
                            st,
                            mybir.ActivationFunctionType.Exp,
                            scale=0.125,
                        )
                        ets[(bi, kc)] = et
                for bi, (q0, qb) in enumerate(Q_BLOCKS):
                    for j0 in range(0, qb // P, 2):
                        js = (j0, j0 + 1)
                        o1 = {}
                        o2 = {}
                        recips = {}
                        for j in js:
                            o1[j] = psa.tile(
                                [P, 512], F32, tag="psa", name=f"o1_{bi}_{j}"
                            )
                            o2[j] = psa.tile(
                                [P, 512], F32, tag="psa", name=f"o2_{bi}_{j}"
                            )
                        for j in js:
                            for kc in range(KC):
                                nc.tensor.matmul(
                                    o1[j],
                                    ets[(bi, kc)][:, j * P : (j + 1) * P],
                                    vp[:, kc, 0:512],
                                    start=(kc == 0),
                                    stop=(kc == KC - 1),
                                )
                        # epilogue part 1 overlaps the run-2 pair (separate
                        # PSUM tiles, so run 2 never waits on ops1 readers)
                        for j in js:
                            recip = smallp.tile([P, 1], F32, tag="recip")
                            nc.vector.reciprocal(recip, o1[j][:, 0:1])
                            recips[j] = recip
                            ob1 = obp.tile([P, 510], F32, tag="ob")
                            nc.vector.tensor_scalar_mul(
                                ob1, o1[j][:, 2:512], recip
                            )
                            qrow = q0 + j * P
                            nc.sync.dma_start(
                                out=out[qrow : qrow + P, 0:510], in_=ob1
                            )
                        for j in js:
                            for kc in range(KC):
                                nc.tensor.matmul(
                                    o2[j][:, 0:258],
                                    ets[(bi, kc)][:, j * P : (j + 1) * P],
                                    vp[:, kc, 512:VW],
                                    start=(kc == 0),
                                    stop=(kc == KC - 1),
                                )
                        for j in js:
                            ob2 = obp.tile([P, 258], F32, tag="ob2")
                            nc.vector.tensor_scalar_mul(
                                ob2, o2[j][:, 0:258], recips[j]
                            )
                            qrow = q0 + j * P
                            nc.sync.dma_start(
                                out=out[qrow : qrow + P, 510:OUT], in_=ob2
                            )
    nc.finalize()
    return nc


_NC_CACHE = None


def _get_nc():
    global _NC_CACHE
    if _NC_CACHE is None:
        _NC_CACHE = build_attention_nc()
    return _NC_CACHE


def make_in_maps(x, kernel):
    x = np.asarray(x, dtype=np.float32)
    k = np.asarray(kernel, dtype=np.float32)
    mt = k[1] @ k[0].T  # M^T = Wk Wq^T, exact fold of the QK^T bilinear form
    w = np.ascontiguousarray(
        np.stack([mt, k[2]]).astype(np.float16)
    )
    in_maps = []
    for core in range(N_CORES):
        b, half = core // 2, core % 2
        xt16 = x[b].T.astype(np.float16)
        xh = np.ascontiguousarray(xt16)
        xq = np.ascontiguousarray(xt16[:, half * NQ : (half + 1) * NQ])
        in_maps.append({"xh": xh, "xq": xq, "w": w})
    return in_maps


def assemble_output(results):
    out = np.empty((B, N, OUT), dtype=np.float32)
    for core in range(N_CORES):
        b, half = core // 2, core % 2
        out[b, half * NQ : (half + 1) * NQ, :] = results[core]["out"]
    return out


def run_on_hw(x, kernel, trace=False):
    nc = _get_nc()
    res = run_bass_kernel_spmd(
        nc, make_in_maps(x, kernel), list(range(N_CORES)), trace=trace
    )
    return assemble_output(res.results), res


def kernel(x, kernel):
    out, _ = run_on_hw(x, kernel, trace=False)
    return out
